# revision 6
# baseline (speedup 1.0000x reference)
"""LSTM decoder kernel for Trainium2 (8 NeuronCores, pure data parallel).

Problem: 25-step autoregressive LSTM decode, BATCH=262144, POSE=16, H=32.
  reference: per step  gates = x@W_ih.T + h@W_hh.T + b;  i,f,g,o = split(gates)
             c = sig(f)*c + sig(i)*tanh(g); h = sig(o)*tanh(c); x = h@W_out.T + b_out

Kernel design (per core, B_loc = 32768 rows):
  * Projection folded into the recurrence:  for t>=1
        gates_t = h_t @ W_eff.T + b_eff,   W_eff = W_ih@W_out + W_hh,
        b_eff = b_ih + b_hh + W_ih@b_out
    so the recurrence needs ONE K=32-per-strip matmul per gate type per step.
  * hidden-on-partitions "strip" layout: h/c live as [128 = 4 strips x 32 hidden,
    batch-cols].  Strip x holds a contiguous 1024-row block of each run's batch.
  * Gate matmuls: BLOCK-DIAGONAL [128,128] lhsT per gate type -> ONE matmul
    instruction computes a gate type for all 4 strips (off-diagonal zeros
    contribute nothing).  Gate tiles are [128, 1024] f32 spanning 2 PSUM banks
    (two N=512 matmuls); ONE ACTIVATE per gate type per step amortizes the
    ~352-cycle ACT fixed cost.  Gate biases ride in ACTIVATE's bias operand.
  * Projection: block-diag W_out lhsT [128, 64] -> X = [64 = 4 strips x 16 pose,
    batch-cols] feature-major PSUM tiles; DVE copies to a bf16 staging buffer;
    one DMA per run writes DRAM; host undoes the strip layout.
  * PSUM: 3 x [128,1024] gate slots (6 banks) + 1 x [64,1024] X slot (2 banks).
  * dtypes: matmuls/h/c/gate activations bf16, PSUM fp32, DRAM out bf16.
"""

import numpy as np
import ml_dtypes

bf16 = ml_dtypes.bfloat16

H = 32
PD = 16
SEQ = 25
BATCH = 262144
NCORES = 8

# per-core decomposition
B_LOC = BATCH // NCORES          # 32768
RUNS = 8
NPACK = 2
NB = 512
C = NPACK * NB                   # 1024 batch cols per strip per run
B_RUN = 4 * C                    # 4096 rows per run


def _f32(x):
    return np.ascontiguousarray(np.asarray(x, dtype=np.float32))


def prep_weights(W_ih, W_hh, b_ih, b_hh, W_out, b_out):
    """Host-side weight preprocessing -> small dram params."""
    W_ih, W_hh, b_ih, b_hh, W_out, b_out = map(
        _f32, (W_ih, W_hh, b_ih, b_hh, W_out, b_out)
    )
    b1 = b_ih + b_hh                       # step-1 bias [4H]
    W_eff = W_ih @ W_out + W_hh            # [4H, H]
    b_eff = b1 + W_ih @ b_out              # [4H]

    def blockdiag4(mat):
        # mat [H, 4H] (K x M_all).  For gate type ty, take M-block
        # mat[:, 32ty:32ty+32] and place it at (32x:32x+32, 32x:32x+32)
        # for each strip x -> [128, 128] block-diagonal lhsT.
        out = np.zeros((4, 128, 128), np.float32)
        for ty in range(4):
            blk = mat[:, 32 * ty : 32 * ty + 32]
            for x in range(4):
                out[ty, 32 * x : 32 * x + 32, 32 * x : 32 * x + 32] = blk
        # concat gate types along cols -> [128, 512]
        return np.ascontiguousarray(
            out.transpose(1, 0, 2).reshape(128, 512).astype(bf16)
        )

    weff = blockdiag4(W_eff.T)
    whh = blockdiag4(W_hh.T)
    wih_pad = np.zeros((H, 4 * H), np.float32)
    wih_pad[:PD] = W_ih.T
    wih = blockdiag4(wih_pad)

    # projection: block-diag W_out.T [32,16] blocks -> [128, 64]
    wout = np.zeros((128, 64), np.float32)
    for x in range(4):
        wout[32 * x : 32 * x + 32, 16 * x : 16 * x + 16] = W_out.T
    wout = np.ascontiguousarray(wout.astype(bf16))

    # bias dram param [128, 8] f32: cols 0-3 = b1 per gate type, 4-7 = b_eff
    bias = np.zeros((128, 8), np.float32)
    for ty in range(4):
        bias[:, ty] = np.tile(b1[32 * ty : 32 * ty + 32], 4)
        bias[:, 4 + ty] = np.tile(b_eff[32 * ty : 32 * ty + 32], 4)
    return dict(weff=weff, whh=whh, wih=wih, wout=wout, bias=bias, b_out=b_out)


def prep_state(arr, runs, npack, feat):
    """[B_loc, feat<=32] batch-major -> strip layout [128, runs*npack*NB] bf16.

    partition 32x+k = feature k of strip x; col r*(npack*NB) + p*NB + j
    = batch row r*B_RUN + x*(npack*NB) + p*NB + j.
    """
    b_loc = arr.shape[0]
    a = np.zeros((b_loc, H), np.float32)
    a[:, : arr.shape[1]] = arr
    a = a.reshape(runs, 4, npack, NB, H)          # [r, x, p, j, k]
    a = a.transpose(1, 4, 0, 2, 3)                # [x, k, r, p, j]
    return np.ascontiguousarray(a.reshape(128, runs * npack * NB).astype(bf16))


def build_nc(runs=RUNS, seq=SEQ):
    import concourse.bass as bass
    import concourse.bacc as bacc
    import concourse.mybir as mybir
    import concourse.tile as tile

    F32 = mybir.dt.float32
    BF16 = mybir.dt.bfloat16
    AF = mybir.ActivationFunctionType
    b_loc = runs * 4 * C

    nc = bacc.Bacc("TRN2", target_bir_lowering=False, debug=False)
    hT_d = nc.declare_dram_parameter("hT", [128, runs * C], BF16, isOutput=False)
    cT_d = nc.declare_dram_parameter("cT", [128, runs * C], BF16, isOutput=False)
    xT_d = nc.declare_dram_parameter("xT", [128, runs * C], BF16, isOutput=False)
    weff_d = nc.declare_dram_parameter("weff", [128, 512], BF16, isOutput=False)
    whh_d = nc.declare_dram_parameter("whh", [128, 512], BF16, isOutput=False)
    wih_d = nc.declare_dram_parameter("wih", [128, 512], BF16, isOutput=False)
    wout_d = nc.declare_dram_parameter("wout", [128, 64], BF16, isOutput=False)
    bias_d = nc.declare_dram_parameter("bias", [128, 8], F32, isOutput=False)
    # feature-major output: part 16x+p = pose p of strip x, cols = run,step,col
    out_d = nc.declare_dram_parameter("out", [64, runs * seq * C], BF16, isOutput=True)

    GATE_FUNC = [AF.Sigmoid, AF.Sigmoid, AF.Tanh, AF.Sigmoid]  # i, f, g, o

    C2X = 2 * C                    # paired-run tile width
    with tile.TileContext(nc) as tc:
        with (
            tc.tile_pool(name="const", bufs=1) as const,
            tc.tile_pool(name="state", bufs=4) as state,
            tc.tile_pool(name="sig", bufs=2) as sig,
            tc.tile_pool(name="gpsum", bufs=2, space=bass.MemorySpace.PSUM) as gpsum,
        ):
            weff_t = const.tile([128, 512], BF16)
            whh_t = const.tile([128, 512], BF16)
            wih_t = const.tile([128, 512], BF16)
            wout_t = const.tile([128, 64], BF16)
            bias_t = const.tile([128, 8], F32)
            nc.sync.dma_start(weff_t[:], weff_d[:])
            nc.sync.dma_start(whh_t[:], whh_d[:])
            nc.sync.dma_start(wih_t[:], wih_d[:])
            nc.sync.dma_start(wout_t[:], wout_d[:])
            nc.sync.dma_start(bias_t[:], bias_d[:])

            # runs fused two-per-tile (halves of [128, 2C] tiles -> one
            # ACTIVATE covers both runs), and two such pairs interleaved so
            # one pair's matmuls/ACTs fill the other's cell-update tail.
            for grp in range(runs // 4):
                st = {}
                for pp in (0, 1):
                    pr = 2 * grp + pp          # pair index; runs 2*pr, 2*pr+1
                    h_sb = state.tile([128, C2X], BF16, tag="h", name=f"h{pr}")
                    c_sb = state.tile([128, C2X], BF16, tag="c", name=f"c{pr}")
                    x0_sb = state.tile([128, C2X], BF16, tag="x0", name=f"x{pr}")
                    nc.sync.dma_start(h_sb[:], hT_d[:, 2 * pr * C : 2 * (pr + 1) * C])
                    nc.sync.dma_start(c_sb[:], cT_d[:, 2 * pr * C : 2 * (pr + 1) * C])
                    nc.sync.dma_start(x0_sb[:], xT_d[:, 2 * pr * C : 2 * (pr + 1) * C])
                    st[pp] = (h_sb, c_sb, x0_sb, pr)

                for t_p in [(t, pp) for t in range(seq) for pp in (0, 1)]:
                    t, pp = t_p
                    h_sb, c_sb, x0_sb, pr = st[pp]
                    step0 = t == 0
                    biascol = 0 if step0 else 4
                    # ---- gate matmuls: one block-diag MM per (type, bank) ----
                    gb = []
                    for ty in range(4):
                        g_t = gpsum.tile([128, C2X], F32, tag="gb", name=f"g{ty}")
                        ws = slice(128 * ty, 128 * (ty + 1))
                        for p in range(2 * NPACK):
                            cs = slice(p * NB, (p + 1) * NB)
                            if step0:
                                nc.tensor.matmul(
                                    g_t[:, cs], whh_t[:, ws], h_sb[:, cs],
                                    start=True, stop=False,
                                )
                                nc.tensor.matmul(
                                    g_t[:, cs], wih_t[:, ws], x0_sb[:, cs],
                                    start=False, stop=True,
                                )
                            else:
                                nc.tensor.matmul(
                                    g_t[:, cs], weff_t[:, ws], h_sb[:, cs],
                                    start=True, stop=True,
                                )
                        gb.append(g_t)
                    # ---- gate activations (bias folded in) ----
                    S = []
                    for ty in range(4):
                        s_t = sig.tile([128, C2X], BF16, tag=f"s{ty}", name=f"s{ty}")
                        nc.scalar.activation(
                            s_t[:], gb[ty][:],
                            GATE_FUNC[ty],
                            bias=bias_t[:, biascol + ty : biascol + ty + 1],
                        )
                        S.append(s_t)
                    s_i, s_f, s_g, s_o = S
                    # ---- cell update (DVE, bf16 2x) ----
                    t1 = sig.tile([128, C2X], BF16, tag="t1")
                    t2 = sig.tile([128, C2X], BF16, tag="t2")
                    nc.vector.tensor_mul(t1[:], s_f[:], c_sb[:])
                    nc.vector.tensor_mul(t2[:], s_i[:], s_g[:])
                    nc.vector.tensor_add(c_sb[:], t1[:], t2[:])
                    s_tc = sig.tile([128, C2X], BF16, tag="stc")
                    nc.scalar.activation(s_tc[:], c_sb[:], AF.Tanh)
                    nc.vector.tensor_mul(h_sb[:], s_o[:], s_tc[:])
                    # ---- projection: block-diag W_out, feature-major X ----
                    X = gpsum.tile([64, C2X], F32, tag="gb", name="X")
                    for p in range(2 * NPACK):
                        cs = slice(p * NB, (p + 1) * NB)
                        nc.tensor.matmul(
                            X[:, cs], wout_t[:], h_sb[:, cs],
                            start=True, stop=True,
                        )
                    xs = state.tile([64, C2X], BF16, tag="xs", name="xs")
                    nc.vector.tensor_copy(xs[:], X[:])
                    # per-run-half output DMA; dram layout unchanged
                    for rr in (0, 1):
                        r = 2 * pr + rr
                        nc.sync.dma_start(
                            out_d[:, (r * seq + t) * C : (r * seq + t + 1) * C],
                            xs[:, rr * C : (rr + 1) * C],
                        )
    nc.compile()
    return nc


_NC_CACHE = {}


def _get_nc(key=("full",)):
    if key not in _NC_CACHE:
        _NC_CACHE[key] = build_nc()
    return _NC_CACHE[key]


def make_in_maps(inputs):
    """host-side prep: full inputs dict -> (in_maps list per core, b_out)."""
    first_input = _f32(inputs["first_input"])
    h0 = _f32(inputs["h0"])
    c0 = _f32(inputs["c0"])
    w = prep_weights(
        inputs["W_ih"], inputs["W_hh"], inputs["b_ih"], inputs["b_hh"],
        inputs["W_out"], inputs["b_out"],
    )
    shared = dict(
        weff=w["weff"], whh=w["whh"], wih=w["wih"], wout=w["wout"], bias=w["bias"]
    )
    in_maps = []
    for ci in range(NCORES):
        rows = slice(ci * B_LOC, (ci + 1) * B_LOC)
        in_maps.append(dict(
            shared,
            hT=prep_state(h0[rows], RUNS, NPACK, H),
            cT=prep_state(c0[rows], RUNS, NPACK, H),
            xT=prep_state(first_input[rows], RUNS, NPACK, PD),
        ))
    return in_maps, w["b_out"]


def unpack_out(raw):
    """[64, RUNS*SEQ*C] bf16 feature-major -> [B_LOC, SEQ, PD] f32.

    part 16x+p = pose p of strip x; col (r*SEQ + t)*C + j
    -> batch row r*B_RUN + x*C + j, step t, pose p.
    """
    a = np.asarray(raw, dtype=np.float32).reshape(4, PD, RUNS, SEQ, C)
    a = a.transpose(2, 0, 4, 3, 1)                # [r, x, j, t, p]
    return np.ascontiguousarray(a.reshape(B_LOC, SEQ, PD))


def kernel(**inputs) -> np.ndarray:
    from concourse.bass_utils import run_bass_kernel_spmd

    in_maps, b_out = make_in_maps(inputs)
    nc = _get_nc()
    res = run_bass_kernel_spmd(nc, in_maps, core_ids=list(range(NCORES)))
    outs = [unpack_out(res.results[i]["out"]) for i in range(NCORES)]
    full = np.concatenate(outs, axis=0)
    full += b_out[None, None, :]
    return full


if __name__ == "__main__":
    nc = build_nc()
    n = sum(len(b.instructions) for b in nc.m.functions[0].blocks)
    print("built; instructions:", n)


# revision 7
# speedup vs baseline: 1.6075x; 1.6075x over previous
"""LSTM decoder kernel for Trainium2 (8 NeuronCores, pure data parallel).

Problem: 25-step autoregressive LSTM decode, BATCH=262144, POSE=16, H=32.
  reference: per step  gates = x@W_ih.T + h@W_hh.T + b;  i,f,g,o = split(gates)
             c = sig(f)*c + sig(i)*tanh(g); h = sig(o)*tanh(c); x = h@W_out.T + b_out

Kernel design (per core, B_loc = 32768 rows):
  * Projection folded into the recurrence:  for t>=1
        gates_t = h_t @ W_eff.T + b_eff,   W_eff = W_ih@W_out + W_hh,
        b_eff = b_ih + b_hh + W_ih@b_out
    so the recurrence needs ONE K=32-per-strip matmul per gate type per step.
  * hidden-on-partitions "strip" layout: h/c live as [128 = 4 strips x 32 hidden,
    batch-cols].  Strip x holds a contiguous 1024-row block of each run's batch.
  * Gate matmuls: BLOCK-DIAGONAL [128,128] lhsT per gate type -> ONE matmul
    instruction computes a gate type for all 4 strips (off-diagonal zeros
    contribute nothing).  Gate tiles are [128, 1024] f32 spanning 2 PSUM banks
    (two N=512 matmuls); ONE ACTIVATE per gate type per step amortizes the
    ~352-cycle ACT fixed cost.  Gate biases ride in ACTIVATE's bias operand.
  * Projection: block-diag W_out lhsT [128, 64] -> X = [64 = 4 strips x 16 pose,
    batch-cols] feature-major PSUM tiles; DVE copies to a bf16 staging buffer;
    one DMA per run writes DRAM; host undoes the strip layout.
  * PSUM: 3 x [128,1024] gate slots (6 banks) + 1 x [64,1024] X slot (2 banks).
  * dtypes: matmuls/h/c/gate activations bf16, PSUM fp32, DRAM out bf16.
"""

import numpy as np
import ml_dtypes

bf16 = ml_dtypes.bfloat16

H = 32
PD = 16
SEQ = 25
BATCH = 262144
NCORES = 8

# per-core decomposition
B_LOC = BATCH // NCORES          # 32768
RUNS = 8
NPACK = 2
NB = 512
C = NPACK * NB                   # 1024 batch cols per strip per run
B_RUN = 4 * C                    # 4096 rows per run


def _f32(x):
    return np.ascontiguousarray(np.asarray(x, dtype=np.float32))


def prep_weights(W_ih, W_hh, b_ih, b_hh, W_out, b_out):
    """Host-side weight preprocessing -> small dram params."""
    W_ih, W_hh, b_ih, b_hh, W_out, b_out = map(
        _f32, (W_ih, W_hh, b_ih, b_hh, W_out, b_out)
    )
    b1 = b_ih + b_hh                       # step-1 bias [4H]
    W_eff = W_ih @ W_out + W_hh            # [4H, H]
    b_eff = b1 + W_ih @ b_out              # [4H]

    def blockdiag4(mat):
        # mat [H, 4H] (K x M_all).  For gate type ty, take M-block
        # mat[:, 32ty:32ty+32] and place it at (32x:32x+32, 32x:32x+32)
        # for each strip x -> [128, 128] block-diagonal lhsT.
        out = np.zeros((4, 128, 128), np.float32)
        for ty in range(4):
            blk = mat[:, 32 * ty : 32 * ty + 32]
            for x in range(4):
                out[ty, 32 * x : 32 * x + 32, 32 * x : 32 * x + 32] = blk
        # concat gate types along cols -> [128, 512]
        return np.ascontiguousarray(
            out.transpose(1, 0, 2).reshape(128, 512).astype(bf16)
        )

    weff = blockdiag4(W_eff.T)
    whh = blockdiag4(W_hh.T)
    wih_pad = np.zeros((H, 4 * H), np.float32)
    wih_pad[:PD] = W_ih.T
    wih = blockdiag4(wih_pad)

    # projection: block-diag W_out.T [32,16] blocks -> [128, 64]
    wout = np.zeros((128, 64), np.float32)
    for x in range(4):
        wout[32 * x : 32 * x + 32, 16 * x : 16 * x + 16] = W_out.T
    wout = np.ascontiguousarray(wout.astype(bf16))

    # bias dram param [128, 8] f32: cols 0-3 = b1 per gate type, 4-7 = b_eff
    bias = np.zeros((128, 8), np.float32)
    for ty in range(4):
        bias[:, ty] = np.tile(b1[32 * ty : 32 * ty + 32], 4)
        bias[:, 4 + ty] = np.tile(b_eff[32 * ty : 32 * ty + 32], 4)
    return dict(weff=weff, whh=whh, wih=wih, wout=wout, bias=bias, b_out=b_out)


def prep_state(arr, runs, npack, feat):
    """[B_loc, feat<=32] batch-major -> strip layout [128, runs*npack*NB] bf16.

    partition 32x+k = feature k of strip x; col r*(npack*NB) + p*NB + j
    = batch row r*B_RUN + x*(npack*NB) + p*NB + j.
    """
    b_loc = arr.shape[0]
    a = np.zeros((b_loc, H), np.float32)
    a[:, : arr.shape[1]] = arr
    a = a.reshape(runs, 4, npack, NB, H)          # [r, x, p, j, k]
    a = a.transpose(1, 4, 0, 2, 3)                # [x, k, r, p, j]
    return np.ascontiguousarray(a.reshape(128, runs * npack * NB).astype(bf16))


def build_nc(runs=RUNS, seq=SEQ):
    import concourse.bass as bass
    import concourse.bacc as bacc
    import concourse.mybir as mybir
    import concourse.tile as tile

    F32 = mybir.dt.float32
    BF16 = mybir.dt.bfloat16
    AF = mybir.ActivationFunctionType
    b_loc = runs * 4 * C

    nc = bacc.Bacc("TRN2", target_bir_lowering=False, debug=False)
    hT_d = nc.declare_dram_parameter("hT", [128, runs * C], BF16, isOutput=False)
    cT_d = nc.declare_dram_parameter("cT", [128, runs * C], BF16, isOutput=False)
    xT_d = nc.declare_dram_parameter("xT", [128, runs * C], BF16, isOutput=False)
    weff_d = nc.declare_dram_parameter("weff", [128, 512], BF16, isOutput=False)
    whh_d = nc.declare_dram_parameter("whh", [128, 512], BF16, isOutput=False)
    wih_d = nc.declare_dram_parameter("wih", [128, 512], BF16, isOutput=False)
    wout_d = nc.declare_dram_parameter("wout", [128, 64], BF16, isOutput=False)
    bias_d = nc.declare_dram_parameter("bias", [128, 8], F32, isOutput=False)
    # feature-major output: part 16x+p = pose p of strip x, cols = run,step,col
    out_d = nc.declare_dram_parameter("out", [64, runs * seq * C], BF16, isOutput=True)

    GATE_FUNC = [AF.Sigmoid, AF.Sigmoid, AF.Tanh, AF.Sigmoid]  # i, f, g, o

    with tile.TileContext(nc) as tc:
        with (
            tc.tile_pool(name="const", bufs=1) as const,
            tc.tile_pool(name="state", bufs=4) as state,
            tc.tile_pool(name="sig", bufs=2) as sig,
            tc.tile_pool(name="gpsum", bufs=3, space=bass.MemorySpace.PSUM) as gpsum,
            tc.tile_pool(name="xpsum", bufs=1, space=bass.MemorySpace.PSUM) as xpsum,
        ):
            weff_t = const.tile([128, 512], BF16)
            whh_t = const.tile([128, 512], BF16)
            wih_t = const.tile([128, 512], BF16)
            wout_t = const.tile([128, 64], BF16)
            bias_t = const.tile([128, 8], F32)
            nc.sync.dma_start(weff_t[:], weff_d[:])
            nc.sync.dma_start(whh_t[:], whh_d[:])
            nc.sync.dma_start(wih_t[:], wih_d[:])
            nc.sync.dma_start(wout_t[:], wout_d[:])
            nc.sync.dma_start(bias_t[:], bias_d[:])

            # two runs interleaved per pair: run B's matmuls/ACTs fill the
            # dependency gaps of run A's serial cell-update chain.
            for pair in range(runs // 2):
                rs = (2 * pair, 2 * pair + 1)
                st = {}
                for r in rs:
                    h_sb = state.tile([128, C], BF16, tag="h", name=f"h{r}")
                    c_sb = state.tile([128, C], BF16, tag="c", name=f"c{r}")
                    x0_sb = state.tile([128, C], BF16, tag="x0", name=f"x{r}")
                    xs = state.tile([64, seq * C], BF16, tag="xs", name=f"xs{r}", bufs=2)
                    nc.sync.dma_start(h_sb[:], hT_d[:, r * C : (r + 1) * C])
                    nc.sync.dma_start(c_sb[:], cT_d[:, r * C : (r + 1) * C])
                    nc.sync.dma_start(x0_sb[:], xT_d[:, r * C : (r + 1) * C])
                    st[r] = (h_sb, c_sb, x0_sb, xs)

                for t_r in [(t, r) for t in range(seq) for r in rs]:
                    t, r = t_r
                    h_sb, c_sb, x0_sb, xs = st[r]
                    step0 = t == 0
                    biascol = 0 if step0 else 4
                    # ---- gate matmuls: one block-diag MM per (type, bank) ----
                    gb = []
                    for ty in range(4):
                        g_t = gpsum.tile([128, C], F32, tag="gb", name=f"g{ty}")
                        ws = slice(128 * ty, 128 * (ty + 1))
                        for p in range(NPACK):
                            cs = slice(p * NB, (p + 1) * NB)
                            if step0:
                                nc.tensor.matmul(
                                    g_t[:, cs], whh_t[:, ws], h_sb[:, cs],
                                    start=True, stop=False,
                                )
                                nc.tensor.matmul(
                                    g_t[:, cs], wih_t[:, ws], x0_sb[:, cs],
                                    start=False, stop=True,
                                )
                            else:
                                nc.tensor.matmul(
                                    g_t[:, cs], weff_t[:, ws], h_sb[:, cs],
                                    start=True, stop=True,
                                )
                        gb.append(g_t)
                    # ---- gate activations (bias folded in) ----
                    S = []
                    for ty in range(4):
                        s_t = sig.tile([128, C], BF16, tag=f"s{ty}", name=f"s{ty}")
                        nc.scalar.activation(
                            s_t[:], gb[ty][:],
                            GATE_FUNC[ty],
                            bias=bias_t[:, biascol + ty : biascol + ty + 1],
                        )
                        S.append(s_t)
                    s_i, s_f, s_g, s_o = S
                    # ---- cell update (DVE, bf16 2x) ----
                    t1 = sig.tile([128, C], BF16, tag="t1")
                    t2 = sig.tile([128, C], BF16, tag="t2")
                    nc.vector.tensor_mul(t1[:], s_f[:], c_sb[:])
                    nc.vector.tensor_mul(t2[:], s_i[:], s_g[:])
                    nc.vector.tensor_add(c_sb[:], t1[:], t2[:])
                    s_tc = sig.tile([128, C], BF16, tag="stc")
                    nc.scalar.activation(s_tc[:], c_sb[:], AF.Tanh)
                    nc.vector.tensor_mul(h_sb[:], s_o[:], s_tc[:])
                    # ---- projection: block-diag W_out, feature-major X ----
                    X = xpsum.tile([64, C], F32, tag="X", name="X")
                    for p in range(NPACK):
                        cs = slice(p * NB, (p + 1) * NB)
                        nc.tensor.matmul(
                            X[:, cs], wout_t[:], h_sb[:, cs],
                            start=True, stop=True,
                        )
                    nc.vector.tensor_copy(xs[:, t * C : (t + 1) * C], X[:])

                # ---- flush pair output ----
                for r in rs:
                    nc.sync.dma_start(
                        out_d[:, r * seq * C : (r + 1) * seq * C], st[r][3][:]
                    )
    nc.compile()
    return nc


_NC_CACHE = {}


def _get_nc(key=("full",)):
    if key not in _NC_CACHE:
        _NC_CACHE[key] = build_nc()
    return _NC_CACHE[key]


def make_in_maps(inputs):
    """host-side prep: full inputs dict -> (in_maps list per core, b_out)."""
    first_input = _f32(inputs["first_input"])
    h0 = _f32(inputs["h0"])
    c0 = _f32(inputs["c0"])
    w = prep_weights(
        inputs["W_ih"], inputs["W_hh"], inputs["b_ih"], inputs["b_hh"],
        inputs["W_out"], inputs["b_out"],
    )
    shared = dict(
        weff=w["weff"], whh=w["whh"], wih=w["wih"], wout=w["wout"], bias=w["bias"]
    )
    in_maps = []
    for ci in range(NCORES):
        rows = slice(ci * B_LOC, (ci + 1) * B_LOC)
        in_maps.append(dict(
            shared,
            hT=prep_state(h0[rows], RUNS, NPACK, H),
            cT=prep_state(c0[rows], RUNS, NPACK, H),
            xT=prep_state(first_input[rows], RUNS, NPACK, PD),
        ))
    return in_maps, w["b_out"]


def unpack_out(raw):
    """[64, RUNS*SEQ*C] bf16 feature-major -> [B_LOC, SEQ, PD] f32.

    part 16x+p = pose p of strip x; col (r*SEQ + t)*C + j
    -> batch row r*B_RUN + x*C + j, step t, pose p.
    """
    a = np.asarray(raw, dtype=np.float32).reshape(4, PD, RUNS, SEQ, C)
    a = a.transpose(2, 0, 4, 3, 1)                # [r, x, j, t, p]
    return np.ascontiguousarray(a.reshape(B_LOC, SEQ, PD))


def kernel(**inputs) -> np.ndarray:
    from concourse.bass_utils import run_bass_kernel_spmd

    in_maps, b_out = make_in_maps(inputs)
    nc = _get_nc()
    res = run_bass_kernel_spmd(nc, in_maps, core_ids=list(range(NCORES)))
    outs = [unpack_out(res.results[i]["out"]) for i in range(NCORES)]
    full = np.concatenate(outs, axis=0)
    full += b_out[None, None, :]
    return full


if __name__ == "__main__":
    nc = build_nc()
    n = sum(len(b.instructions) for b in nc.m.functions[0].blocks)
    print("built; instructions:", n)
